# revision 56
# baseline (speedup 1.0000x reference)
"""HMLC hierarchical SupCon loss on 8 Trainium2 NeuronCores.

Strategy (data-parallel over anchor rows, exploiting logit symmetry):
  - cf = concat of the two views -> [4096, 768] L2-normalized features,
    pre-scaled by S=64 and quantized to fp8 e4m3 on host.
  - The [4096, 4096] logit matrix is symmetric, so each unordered block
    pair is computed ONCE: the 32 anchor row-blocks are spread circulantly
    over the 8 cores (core c owns blocks (2c + t) mod 32, t in {0,1,16,17});
    block m computes columns [128m, 128m + 17*128) mod 4096, which covers
    every pair at circular block distance < 17 (max distance is 16). Each
    core's cfb input is column-rotated by 256c so the four chunks sit at
    identical local offsets on every core (pure SPMD, no stragglers).
  - Logits come from fp8 DoubleRow matmuls (2 contraction rows/cycle,
    fp32 PSUM accumulate); PSUM is drained as bf16 logits (dot/T) by
    scaled copies alternating between the scalar and vector engines.
  - The host exps the stripes in fp64, mirrors uncovered blocks from the
    transpose (both orientations use identical fp8 operands and k-order,
    so E is exactly symmetric), and does all label-dependent bookkeeping
    (positive masks via class centroids, dedup/valid updates, hmce
    combination) in exact fp64: the device only supplies E for the masked
    softmax denominators. No m* shift is applied on device -- it cancels
    algebraically, and log-denominators absorb it.
"""

import sys

for _p in ("/opt/trn_rl_repo", "/root/.axon_site/_ro/trn_rl_repo"):
    if _p not in sys.path:
        sys.path.append(_p)

import numpy as np
import ml_dtypes

import concourse.bass as bass
import concourse.bacc as bacc
import concourse.tile as tile
import concourse.mybir as mybir
from concourse.bass_utils import run_bass_kernel_spmd

B, V, D = 2048, 2, 768
N = V * B            # 4096 total anchors/contrast columns
NC = 8               # cores
RPC = N // NC        # 512 rows per core
JCH = D // 256       # 3 DoubleRow contraction chunks (256 deep each)
T = 0.07
FP8_SCALE = 64.0     # pre-scale before e4m3 quantization (keeps values normal)
ESCALE = 1.0 / (FP8_SCALE * FP8_SCALE * T)

# Circulant-symmetric block assignment: core c owns anchor row-blocks
# (2c + t) mod 32 for t in CHUNK_TS, sitting at local block t after the
# host rotates that core's cfb columns by 256c. Chunk t must cover local
# columns [128t, 128t + 17*128) mod 4096 (17 blocks incl. itself) -- the
# exact region lists below do that with one leading alignment block for
# t = 1/17. Shared by the device build and the host reassembly.
CHUNK_TS = (0, 1, 16, 17)
CHUNK_REGIONS = [
    [(0, 512), (512, 1024), (1024, 1536), (1536, 2048), (2048, 2176)],
    [(0, 512), (512, 1024), (1024, 1536), (1536, 2048), (2048, 2304)],
    [(0, 128), (2048, 2560), (2560, 3072), (3072, 3584), (3584, 4096)],
    [(0, 256), (2048, 2560), (2560, 3072), (3072, 3584), (3584, 4096)],
]
CHUNK_STORES = [
    [(0, 1024), (1024, 2048), (2048, 2176)],
    [(0, 1024), (1024, 2048), (2048, 2304)],
    [(0, 128), (2048, 3072), (3072, 4096)],
    [(0, 256), (2048, 3072), (3072, 4096)],
]

_PROGRAM = None


def _build_program():
    nc = bacc.Bacc("TRN2", target_bir_lowering=False, debug=False, num_devices=NC)

    f8 = mybir.dt.float8e4
    cfb = nc.declare_dram_parameter("cfb", [D, N], f8, isOutput=False)
    # anc carries the same bytes as a [D, RPC] row-major array, but declared
    # [128, 6*RPC] so the pair-pack load below is one 3KB-per-partition DMA.
    anc = nc.declare_dram_parameter("anc", [128, (D // 128) * RPC], f8,
                                    isOutput=False)
    eout = nc.declare_dram_parameter("eout", [RPC, N], mybir.dt.bfloat16, isOutput=True)

    DR = mybir.MatmulPerfMode.DoubleRow

    with tile.TileContext(nc) as tc:
        with (
            tc.tile_pool(name="cf", bufs=1) as cfp,
            tc.tile_pool(name="an", bufs=1) as anp_,
            tc.tile_pool(name="ps", bufs=8, space="PSUM") as psp,
            tc.tile_pool(name="e", bufs=4) as ep,
        ):
            # DoubleRow pack layout: tile [128, 6, F]; partition p holds
            # contraction rows 6p..6p+5 (six consecutive 4KB DRAM rows ->
            # one contiguous partition line). Matmul j contracts the
            # [:, 2j:2j+2, :] pair. Any consistent k permutation is fine
            # since both operands use the same one.
            cft = cfp.tile([128, JCH * 2, N], f8, tag="cf", name="cft")
            ant = anp_.tile([128, JCH * 2, RPC], f8, tag="an", name="ant")
            # anchors on the ACT hardware-DGE queue so they stream in
            # PARALLEL with the cfb pieces on the SP queue -- the cfb stream
            # (which paces the whole kernel) starts ~1.7us earlier. ant is a
            # separate tile so weight loads don't contend with rhs streaming
            # from cft. cfb pieces ordered to match consumption.
            nc.scalar.dma_start(ant, anc[:, :])
            for lo, hi in ((0, 512), (512, 1024), (1024, 2048),
                           (2048, 3072), (3072, 4096)):
                nc.sync.dma_start(cft[:, :, lo:hi], cfb[:, lo:hi])

            # HAM warm-up: dummy matmuls on a raw (uninitialized) SBUF
            # scratch keep the PE busy through the preamble/DMA window so
            # real matmuls start at full clock. Garbage values are fine:
            # ps_warm is never read (real groups reset PSUM via start=True),
            # and skipping the memset removes every cross-engine dependency.
            sc = nc.alloc_sbuf_tensor("warm_sc", [128, 2, 640], f8).ap()
            ps_warm = psp.tile([128, 512], mybir.dt.float32, tag="ps", name="ps_warm")
            for _ in range(12):
                nc.tensor.matmul(ps_warm, sc[:, :, 0:128],
                                 sc[:, :, 128:640], start=True, stop=True,
                                 perf_mode=DR)

            # Circulant-symmetric coverage: this core owns anchor row-blocks
            # t in CHUNK_TS (local block ids; global chunk = (2c + t) mod 32
            # after the host's per-core column rotation by 256c). Chunk t
            # computes E[t-block rows, local cols 512-slabs] for the slabs in
            # SLABS[t] -- together the 32 chunks cover every unordered block
            # pair once (circular distance < 17), so the host mirrors the
            # rest. All drains emit bf16 LOGITS (dot/T; scaled copies on
            # alternating ACT/DVE -- no exp table needed), host exps them.
            # PSUM: [128, 512] regions (1 bank), bufs=8 pipelining.
            ets = [ep.tile([128, N], mybir.dt.bfloat16, tag="e", name=f"et{k}")
                   for k in range(4)]
            # Per-chunk (k) exact column regions (local), processed in
            # input-arrival order; tails are narrower than 512 so no junk
            # columns beyond the 17-block circulant span (plus one leading
            # alignment block for t=1/17). Stores fire once every region
            # inside the store range has drained.
            drained = [set() for _ in range(4)]
            rctr = 0
            flat = sorted(
                ((lo // 512, k, lo, hi)
                 for k in range(4) for lo, hi in CHUNK_REGIONS[k]))
            for _, k, lo, hi in flat:
                et = ets[k]
                w = hi - lo
                ps = psp.tile([128, 512], mybir.dt.float32, tag="ps",
                              name=f"ps{k}_{lo}")
                for j in range(JCH):
                    nc.tensor.matmul(
                        ps[:, 0:w],
                        ant[:, 2 * j:2 * (j + 1), 128 * k:128 * (k + 1)],
                        cft[:, 2 * j:2 * (j + 1), lo:hi],
                        start=(j == 0),
                        stop=(j == JCH - 1),
                        perf_mode=DR,
                    )
                if rctr % 2 == 0:
                    nc.scalar.mul(et[:, lo:hi], ps[:, 0:w], ESCALE)
                else:
                    nc.vector.tensor_scalar_mul(et[:, lo:hi], ps[:, 0:w],
                                                ESCALE)
                rctr += 1
                drained[k].add((lo, hi))
                for slo, shi in CHUNK_STORES[k]:
                    if slo <= lo and hi <= shi:
                        done = sum(b - a for a, b in drained[k]
                                   if slo <= a and b <= shi)
                        if done == shi - slo:
                            nc.sync.dma_start(
                                eout[128 * k:128 * (k + 1), slo:shi],
                                et[:, slo:shi])
    nc.compile()
    return nc


def _get_program():
    global _PROGRAM
    if _PROGRAM is None:
        _PROGRAM = _build_program()
    return _PROGRAM


def _run_device(features, trace=False):
    """features: [B, 2, D] fp32. Returns (E [N, N] fp32, BassKernelResults)."""
    cf = features.transpose(1, 0, 2).reshape(N, D)
    cfq = (cf * FP8_SCALE).astype(ml_dtypes.float8_e4m3)
    cfT = np.ascontiguousarray(cfq.T)  # [D, N] fp8
    nc = _get_program()
    # Core c's cfb is column-rotated by 256c, so its 4 anchor chunks
    # (global 128-row blocks (2c + t) mod 32, t in CHUNK_TS) sit at local
    # blocks t. anc packs the 4 chunks' anchor columns contiguously.
    in_maps = []
    for c in range(NC):
        ms = [(2 * c + t) % 32 for t in CHUNK_TS]
        ancc = np.concatenate(
            [cfT[:, 128 * mk:128 * (mk + 1)] for mk in ms], axis=1)
        in_maps.append({
            "cfb": np.ascontiguousarray(np.roll(cfT, -256 * c, axis=1)),
            "anc": np.ascontiguousarray(ancc).reshape(128, -1),
        })
    res = run_bass_kernel_spmd(nc, in_maps, list(range(NC)), trace=trace)
    # Reassemble: each stored slab holds bf16 LOGITS of
    # E[chunk rows, local cols]; local col x <-> global (x + 256c) % 4096.
    # Fill covered blocks, then mirror the rest (E is exactly symmetric:
    # both orientations use identical fp8 operands and k-order).
    E = np.zeros((N, N), dtype=np.float64)
    bmask = np.zeros((32, 32), dtype=bool)
    for c in range(NC):
        eo = res.results[c]["eout"].astype(np.float64)
        for k, t in enumerate(CHUNK_TS):
            mk = (2 * c + t) % 32
            rows = slice(128 * mk, 128 * (mk + 1))
            for lo, hi in CHUNK_REGIONS[k]:
                gidx = (lo + 256 * c + np.arange(hi - lo)) % N
                E[rows, gidx] = np.exp(eo[128 * k:128 * (k + 1), lo:hi])
                for bb in range((hi - lo) // 128):
                    bmask[mk, (gidx[0] // 128 + bb) % 32] = True
    for a in range(32):
        for b in range(32):
            if not bmask[a, b]:
                E[128 * a:128 * (a + 1), 128 * b:128 * (b + 1)] = \
                    E[128 * b:128 * (b + 1), 128 * a:128 * (a + 1)].T
    return E, res


def _host_postprocess(E, features, labels):
    """Combine device denominators with exact host positive-pair sums."""
    L = labels.shape[1]
    f = features.astype(np.float64)
    labels = np.asarray(labels)
    normsq = np.einsum("bvd,bvd->bv", f, f)           # [B, 2]
    cross = np.einsum("bd,bd->b", f[:, 0], f[:, 1])   # [B]
    fsum = f.sum(axis=1)                               # [B, D]

    E = E.astype(np.float64)
    diagE = np.diagonal(E).copy()

    idx = np.arange(B)
    valid = np.ones(B, dtype=bool)
    cum = 0.0
    nlayers = 0.0
    max_lower = -np.inf

    for layer_offset in range(1, L):
        tcol = L - layer_offset - 1
        v = labels[:, tcol]
        nz = v != 0
        active = bool(np.any(nz & valid))

        colv = np.concatenate([valid, valid]).astype(np.float64)
        denom = E @ colv - diagE * colv   # masked row-sum, self-excluded

        sel = valid & nz
        nlab = int(v.max()) + 1
        Wsum = np.zeros((nlab, D))
        np.add.at(Wsum, v[sel], fsum[sel])
        K = np.bincount(v[sel], minlength=nlab).astype(np.float64)

        validf = valid.astype(np.float64)
        P = np.zeros((V, B))
        n = np.zeros((V, B))
        for w in range(V):
            dotW = np.einsum("bd,bd->b", f[:, w], Wsum[v])
            P[w] = np.where(nz, (dotW - validf * normsq[:, w]) / T,
                            validf * cross / T)
            n[w] = np.where(nz, 2.0 * K[v] - validf, validf)
        P = P.reshape(N)
        n = n.reshape(N)

        n_c = np.where(n < 1e-6, 1.0, n)
        # E' = exp(dot/T) (no m* shift on device), so log(denom') already
        # includes the m* term of the reference's shifted softmax.
        logden = np.log(np.where(denom > 0, denom, 1.0))
        mlpp = (P - n * logden) / n_c
        loss_per = -mlpp

        valid2 = np.concatenate([valid, valid])
        nvalid = float(valid.sum())
        layer_loss = float(np.sum(np.where(valid2, loss_per, 0.0)) / (V * nvalid))

        ll = max(max_lower, layer_loss)
        penalty = 2.0 ** (1.0 / layer_offset)
        if active:
            cum += penalty * ll
            nlayers += 1.0
            max_lower = max(max_lower, ll)
            nzv = nz & valid
            same = (v[:, None] == v[None, :]) & nzv[:, None] & nzv[None, :]
            earlier = same & (idx[None, :] < idx[:, None])
            is_first = ~np.any(earlier, axis=1)
            valid = valid & ((v == 0) | is_first)

    return np.float32(cum / nlayers)


def kernel(features, labels):
    features = np.asarray(features, dtype=np.float32)
    labels = np.asarray(labels)
    E, _ = _run_device(features)
    return _host_postprocess(E, features, labels)


def kernel_traced(features, labels):
    """Like kernel() but also returns the BassKernelResults (for profiling)."""
    features = np.asarray(features, dtype=np.float32)
    labels = np.asarray(labels)
    E, res = _run_device(features, trace=True)
    return _host_postprocess(E, features, labels), res


# revision 58
# speedup vs baseline: 1.1872x; 1.1872x over previous
"""HMLC hierarchical SupCon loss on 8 Trainium2 NeuronCores.

Strategy (data-parallel over anchor rows, exploiting logit symmetry):
  - cf = concat of the two views -> [4096, 768] L2-normalized features,
    pre-scaled by S=64 and quantized to fp8 e4m3 on host.
  - The [4096, 4096] logit matrix is symmetric, so each unordered block
    pair is computed ONCE: the 32 anchor row-blocks are spread circulantly
    over the 8 cores (core c owns blocks (2c + t) mod 32, t in {0,1,16,17});
    block m computes columns [128m, 128m + 17*128) mod 4096, which covers
    every pair at circular block distance < 17 (max distance is 16). Each
    core's cfb input is column-rotated by 256c so the four chunks sit at
    identical local offsets on every core (pure SPMD, no stragglers).
  - Logits come from fp8 DoubleRow matmuls (2 contraction rows/cycle,
    fp32 PSUM accumulate); PSUM is drained as bf16 logits (dot/T) by
    scaled copies alternating between the scalar and vector engines.
  - The host exps the stripes in fp64, mirrors uncovered blocks from the
    transpose (both orientations use identical fp8 operands and k-order,
    so E is exactly symmetric), and does all label-dependent bookkeeping
    (positive masks via class centroids, dedup/valid updates, hmce
    combination) in exact fp64: the device only supplies E for the masked
    softmax denominators. No m* shift is applied on device -- it cancels
    algebraically, and log-denominators absorb it.
"""

import sys

for _p in ("/opt/trn_rl_repo", "/root/.axon_site/_ro/trn_rl_repo"):
    if _p not in sys.path:
        sys.path.append(_p)

import numpy as np
import ml_dtypes

import concourse.bass as bass
import concourse.bacc as bacc
import concourse.tile as tile
import concourse.mybir as mybir
from concourse.bass_utils import run_bass_kernel_spmd

B, V, D = 2048, 2, 768
N = V * B            # 4096 total anchors/contrast columns
NC = 8               # cores
RPC = N // NC        # 512 rows per core
JCH = D // 256       # 3 DoubleRow contraction chunks (256 deep each)
T = 0.07
FP8_SCALE = 64.0     # pre-scale before e4m3 quantization (keeps values normal)
ESCALE = 1.0 / (FP8_SCALE * FP8_SCALE * T)

# Circulant-symmetric block assignment: core c owns anchor row-blocks
# (2c + t) mod 32 for t in CHUNK_TS, sitting at local block t after the
# host rotates that core's cfb columns by 256c. Chunk t must cover local
# columns [128t, 128t + 17*128) mod 4096 (17 blocks incl. itself) -- the
# exact region lists below do that with one leading alignment block for
# t = 1/17. Shared by the device build and the host reassembly.
CHUNK_TS = (0, 1, 16, 17)
CHUNK_REGIONS = [
    [(0, 512), (512, 1024), (1024, 1536), (1536, 2048), (2048, 2176)],
    [(0, 512), (512, 1024), (1024, 1536), (1536, 2048), (2048, 2304)],
    [(0, 128), (2048, 2560), (2560, 3072), (3072, 3584), (3584, 4096)],
    [(0, 256), (2048, 2560), (2560, 3072), (3072, 3584), (3584, 4096)],
]
CHUNK_STORES = [
    [(0, 1024), (1024, 2048), (2048, 2176)],
    [(0, 1024), (1024, 2048), (2048, 2304)],
    [(0, 128), (2048, 3072), (3072, 4096)],
    [(0, 256), (2048, 3072), (3072, 4096)],
]

_PROGRAM = None


def _build_program():
    nc = bacc.Bacc("TRN2", target_bir_lowering=False, debug=False, num_devices=NC)

    f8 = mybir.dt.float8e4
    cfb = nc.declare_dram_parameter("cfb", [D, N], f8, isOutput=False)
    # anc carries the same bytes as a [D, RPC] row-major array, but declared
    # [128, 6*RPC] so the pair-pack load below is one 3KB-per-partition DMA.
    anc = nc.declare_dram_parameter("anc", [128, (D // 128) * RPC], f8,
                                    isOutput=False)
    eout = nc.declare_dram_parameter("eout", [RPC, N], mybir.dt.bfloat16, isOutput=True)

    DR = mybir.MatmulPerfMode.DoubleRow

    with tile.TileContext(nc) as tc:
        with (
            tc.tile_pool(name="cf", bufs=1) as cfp,
            tc.tile_pool(name="an", bufs=1) as anp_,
            tc.tile_pool(name="ps", bufs=8, space="PSUM") as psp,
            tc.tile_pool(name="e", bufs=4) as ep,
        ):
            # DoubleRow pack layout: tile [128, 6, F]; partition p holds
            # contraction rows 6p..6p+5 (six consecutive 4KB DRAM rows ->
            # one contiguous partition line). Matmul j contracts the
            # [:, 2j:2j+2, :] pair. Any consistent k permutation is fine
            # since both operands use the same one.
            cft = cfp.tile([128, JCH * 2, N], f8, tag="cf", name="cft")
            ant = anp_.tile([128, JCH * 2, RPC], f8, tag="an", name="ant")
            # anchors on the ACT hardware-DGE queue so they stream in
            # PARALLEL with the cfb pieces on the SP queue -- the cfb stream
            # (which paces the whole kernel) starts ~1.7us earlier. ant is a
            # separate tile so weight loads don't contend with rhs streaming
            # from cft. cfb pieces ordered to match consumption.
            nc.scalar.dma_start(ant, anc[:, :])
            for lo, hi in ((0, 512), (512, 1024), (1024, 2048),
                           (2048, 3072), (3072, 4096)):
                nc.sync.dma_start(cft[:, :, lo:hi], cfb[:, lo:hi])

            # HAM warm-up: dummy matmuls on a raw (uninitialized) SBUF
            # scratch keep the PE busy through the preamble/DMA window so
            # real matmuls start at full clock. Garbage values are fine:
            # ps_warm is never read (real groups reset PSUM via start=True),
            # and skipping the memset removes every cross-engine dependency.
            sc = nc.alloc_sbuf_tensor("warm_sc", [128, 2, 640], f8).ap()
            ps_warm = psp.tile([128, 512], mybir.dt.float32, tag="ps", name="ps_warm")
            for _ in range(12):
                nc.tensor.matmul(ps_warm, sc[:, :, 0:128],
                                 sc[:, :, 128:640], start=True, stop=True,
                                 perf_mode=DR)

            # Circulant-symmetric coverage: this core owns anchor row-blocks
            # t in CHUNK_TS (local block ids; global chunk = (2c + t) mod 32
            # after the host's per-core column rotation by 256c). Chunk t
            # computes E[t-block rows, local cols 512-slabs] for the slabs in
            # SLABS[t] -- together the 32 chunks cover every unordered block
            # pair once (circular distance < 17), so the host mirrors the
            # rest. All drains emit bf16 LOGITS (dot/T; scaled copies on
            # alternating ACT/DVE -- no exp table needed), host exps them.
            # PSUM: [128, 512] regions (1 bank), bufs=8 pipelining.
            ets = [ep.tile([128, N], mybir.dt.bfloat16, tag="e", name=f"et{k}")
                   for k in range(4)]
            # Per-chunk (k) exact column regions (local), processed in
            # input-arrival order; tails are narrower than 512 so no junk
            # columns beyond the 17-block circulant span (plus one leading
            # alignment block for t=1/17). Stores fire once every region
            # inside the store range has drained.
            drained = [set() for _ in range(4)]
            rctr = 0
            flat = sorted(
                ((lo // 512, k, lo, hi)
                 for k in range(4) for lo, hi in CHUNK_REGIONS[k]))
            for fi, (_, k, lo, hi) in enumerate(flat):
                et = ets[k]
                w = hi - lo
                ps = psp.tile([128, 512], mybir.dt.float32, tag="ps",
                              name=f"ps{k}_{lo}")
                for j in range(JCH):
                    nc.tensor.matmul(
                        ps[:, 0:w],
                        ant[:, 2 * j:2 * (j + 1), 128 * k:128 * (k + 1)],
                        cft[:, 2 * j:2 * (j + 1), lo:hi],
                        start=(j == 0),
                        stop=(j == JCH - 1),
                        perf_mode=DR,
                    )
                if fi == len(flat) - 1:
                    # Final region: split the drain across BOTH engines in
                    # parallel halves -- it sits serially after the last
                    # matmul, so halving it shortens the kernel tail.
                    h = w // 2
                    nc.scalar.mul(et[:, lo:lo + h], ps[:, 0:h], ESCALE)
                    nc.vector.tensor_scalar_mul(et[:, lo + h:hi],
                                                ps[:, h:w], ESCALE)
                elif rctr % 2 == 0:
                    nc.scalar.mul(et[:, lo:hi], ps[:, 0:w], ESCALE)
                else:
                    nc.vector.tensor_scalar_mul(et[:, lo:hi], ps[:, 0:w],
                                                ESCALE)
                rctr += 1
                drained[k].add((lo, hi))
                for slo, shi in CHUNK_STORES[k]:
                    if slo <= lo and hi <= shi:
                        done = sum(b - a for a, b in drained[k]
                                   if slo <= a and b <= shi)
                        if done == shi - slo:
                            nc.sync.dma_start(
                                eout[128 * k:128 * (k + 1), slo:shi],
                                et[:, slo:shi])
    nc.compile()
    return nc


def _get_program():
    global _PROGRAM
    if _PROGRAM is None:
        _PROGRAM = _build_program()
    return _PROGRAM


def _run_device(features, trace=False):
    """features: [B, 2, D] fp32. Returns (E [N, N] fp32, BassKernelResults)."""
    cf = features.transpose(1, 0, 2).reshape(N, D)
    cfq = (cf * FP8_SCALE).astype(ml_dtypes.float8_e4m3)
    cfT = np.ascontiguousarray(cfq.T)  # [D, N] fp8
    nc = _get_program()
    # Core c's cfb is column-rotated by 256c, so its 4 anchor chunks
    # (global 128-row blocks (2c + t) mod 32, t in CHUNK_TS) sit at local
    # blocks t. anc packs the 4 chunks' anchor columns contiguously.
    in_maps = []
    for c in range(NC):
        ms = [(2 * c + t) % 32 for t in CHUNK_TS]
        ancc = np.concatenate(
            [cfT[:, 128 * mk:128 * (mk + 1)] for mk in ms], axis=1)
        in_maps.append({
            "cfb": np.ascontiguousarray(np.roll(cfT, -256 * c, axis=1)),
            "anc": np.ascontiguousarray(ancc).reshape(128, -1),
        })
    res = run_bass_kernel_spmd(nc, in_maps, list(range(NC)), trace=trace)
    # Reassemble: each stored slab holds bf16 LOGITS of
    # E[chunk rows, local cols]; local col x <-> global (x + 256c) % 4096.
    # Fill covered blocks, then mirror the rest (E is exactly symmetric:
    # both orientations use identical fp8 operands and k-order).
    E = np.zeros((N, N), dtype=np.float64)
    bmask = np.zeros((32, 32), dtype=bool)
    for c in range(NC):
        eo = res.results[c]["eout"].astype(np.float64)
        for k, t in enumerate(CHUNK_TS):
            mk = (2 * c + t) % 32
            rows = slice(128 * mk, 128 * (mk + 1))
            for lo, hi in CHUNK_REGIONS[k]:
                gidx = (lo + 256 * c + np.arange(hi - lo)) % N
                E[rows, gidx] = np.exp(eo[128 * k:128 * (k + 1), lo:hi])
                for bb in range((hi - lo) // 128):
                    bmask[mk, (gidx[0] // 128 + bb) % 32] = True
    for a in range(32):
        for b in range(32):
            if not bmask[a, b]:
                E[128 * a:128 * (a + 1), 128 * b:128 * (b + 1)] = \
                    E[128 * b:128 * (b + 1), 128 * a:128 * (a + 1)].T
    return E, res


def _host_postprocess(E, features, labels):
    """Combine device denominators with exact host positive-pair sums."""
    L = labels.shape[1]
    f = features.astype(np.float64)
    labels = np.asarray(labels)
    normsq = np.einsum("bvd,bvd->bv", f, f)           # [B, 2]
    cross = np.einsum("bd,bd->b", f[:, 0], f[:, 1])   # [B]
    fsum = f.sum(axis=1)                               # [B, D]

    E = E.astype(np.float64)
    diagE = np.diagonal(E).copy()

    idx = np.arange(B)
    valid = np.ones(B, dtype=bool)
    cum = 0.0
    nlayers = 0.0
    max_lower = -np.inf

    for layer_offset in range(1, L):
        tcol = L - layer_offset - 1
        v = labels[:, tcol]
        nz = v != 0
        active = bool(np.any(nz & valid))

        colv = np.concatenate([valid, valid]).astype(np.float64)
        denom = E @ colv - diagE * colv   # masked row-sum, self-excluded

        sel = valid & nz
        nlab = int(v.max()) + 1
        Wsum = np.zeros((nlab, D))
        np.add.at(Wsum, v[sel], fsum[sel])
        K = np.bincount(v[sel], minlength=nlab).astype(np.float64)

        validf = valid.astype(np.float64)
        P = np.zeros((V, B))
        n = np.zeros((V, B))
        for w in range(V):
            dotW = np.einsum("bd,bd->b", f[:, w], Wsum[v])
            P[w] = np.where(nz, (dotW - validf * normsq[:, w]) / T,
                            validf * cross / T)
            n[w] = np.where(nz, 2.0 * K[v] - validf, validf)
        P = P.reshape(N)
        n = n.reshape(N)

        n_c = np.where(n < 1e-6, 1.0, n)
        # E' = exp(dot/T) (no m* shift on device), so log(denom') already
        # includes the m* term of the reference's shifted softmax.
        logden = np.log(np.where(denom > 0, denom, 1.0))
        mlpp = (P - n * logden) / n_c
        loss_per = -mlpp

        valid2 = np.concatenate([valid, valid])
        nvalid = float(valid.sum())
        layer_loss = float(np.sum(np.where(valid2, loss_per, 0.0)) / (V * nvalid))

        ll = max(max_lower, layer_loss)
        penalty = 2.0 ** (1.0 / layer_offset)
        if active:
            cum += penalty * ll
            nlayers += 1.0
            max_lower = max(max_lower, ll)
            nzv = nz & valid
            same = (v[:, None] == v[None, :]) & nzv[:, None] & nzv[None, :]
            earlier = same & (idx[None, :] < idx[:, None])
            is_first = ~np.any(earlier, axis=1)
            valid = valid & ((v == 0) | is_first)

    return np.float32(cum / nlayers)


def kernel(features, labels):
    features = np.asarray(features, dtype=np.float32)
    labels = np.asarray(labels)
    E, _ = _run_device(features)
    return _host_postprocess(E, features, labels)


def kernel_traced(features, labels):
    """Like kernel() but also returns the BassKernelResults (for profiling)."""
    features = np.asarray(features, dtype=np.float32)
    labels = np.asarray(labels)
    E, res = _run_device(features, trace=True)
    return _host_postprocess(E, features, labels), res
